# revision 5
# baseline (speedup 1.0000x reference)
"""Trainium2 Bass kernel for a 2-layer GCN (LinkPredictionGNN encoder).

Computation (per reference):
    z = GCNConv(relu(GCNConv(x, W1, b1)), W2, b2)
where GCNConv adds self-loops and uses symmetric D^-1/2 (A+I) D^-1/2
normalization.

Distribution strategy (8 NeuronCores, SPMD single NEFF):
  * Nodes are sharded contiguously: core c owns nodes [c*6250, (c+1)*6250).
  * Each core computes H = x_own @ W, scales rows by dinv (=1/sqrt(deg)),
    and the per-core shards are AllGather'd into a full node-feature table
    in each core's DRAM.
  * Edges are partitioned by destination owner.  Per destination tile of
    128 nodes, messages are gathered by src row with the SWDGE dma_gather
    instruction (per-edge rows from the DRAM table into SBUF, edge on
    partition), and segment-summed into PSUM with one-hot matmuls on the
    tensor engine (lhsT[e, j] = (dst_local[e] == j)).
  * Epilogue per tile: z = dinv * (acc + G_own) + b  (the G_own term is the
    self-loop dinv^2 * h), relu, then the layer-2 transform z1 @ W2 (via a
    PE transpose) feeding the second AllGather + message passing round.

dma_gather indices are int16, so the 50176-row table is addressed in two
halves (lo/hi) of 25088 rows; each destination tile's edge list is split by
source half and padded to a whole number of 128-edge tiles.  Padded edges
use dst_local = -1 so their one-hot column is all-zero (they contribute
nothing regardless of what row they gather).
"""

import os
import sys

import numpy as np

if "/opt/trn_rl_repo" not in sys.path:
    sys.path.insert(0, "/opt/trn_rl_repo")

LAST_RESULTS = None  # BassKernelResults of the most recent kernel() call


# ----------------------------------------------------------------------------
# configuration
# ----------------------------------------------------------------------------
class Cfg:
    def __init__(self, n_nodes, n_edges, cin, chid, cout, cores=8,
                 nodes_per_core=None, group=3):
        self.N = n_nodes
        self.E = n_edges
        self.CIN = cin
        self.CHID = chid
        self.COUT = cout
        self.CORES = cores
        self.NPC = nodes_per_core or -(-n_nodes // cores)
        assert self.NPC * cores >= n_nodes
        self.TILES = -(-self.NPC // 128)
        self.SLOTS = self.TILES * 128
        self.TOTAL = self.SLOTS * cores
        assert cores % 2 == 0
        self.HALF = self.TOTAL // 2
        assert self.HALF <= 32767, "table half must fit int16 indices"
        self.DEAD = self.SLOTS - self.NPC
        self.GROUP = group  # dst tiles per dma_gather chunk


REAL = Cfg(n_nodes=50000, n_edges=800000, cin=128, chid=128, cout=64)


# ----------------------------------------------------------------------------
# host-side graph partitioning / data staging (numpy only, no float math on x)
# ----------------------------------------------------------------------------
def _wrap_idxs(a):
    """[n] int array (n % 128 == 0) -> SWDGE idx layout [128, n//16] int16.

    idx i lives at [i % 16, i // 16], replicated across the 8 groups of 16
    partitions (one per GPSIMD Q7 core).
    """
    assert len(a) % 128 == 0
    w = np.ascontiguousarray(a.reshape(-1, 16).T.astype(np.int16))
    return np.tile(w, (8, 1))


def prep_inputs(cfg, x, edge_index, W1, b1, W2, b2):
    """Returns (in_maps, meta). meta holds the per-tile edge-tile counts
    (identical across cores) the device program is specialized on."""
    N, NPC, SLOTS, TILES, HALF = cfg.N, cfg.NPC, cfg.SLOTS, cfg.TILES, cfg.HALF
    CORES = cfg.CORES

    src = np.asarray(edge_index[0], dtype=np.int64)
    dst = np.asarray(edge_index[1], dtype=np.int64)

    deg = np.bincount(dst, minlength=N).astype(np.float32) + 1.0  # incl self-loop

    # node v -> table row (identity sharding with per-core dead tail slots)
    src_row = src + cfg.DEAD * (src // NPC)
    half_flag = src_row >= HALF
    rel_row = src_row - half_flag * HALF

    core_of = dst // NPC
    within = dst % NPC
    tile_of = within // 128
    slot_of = within % 128

    # bucket edges: [core][tile] -> (rel_rows, slots) split by half
    # sort once by (core, tile, half) for cheap grouping
    order = np.lexsort((half_flag, tile_of, core_of))
    s_core = core_of[order]
    s_tile = tile_of[order]
    s_half = half_flag[order]
    s_rel = rel_row[order]
    s_slot = slot_of[order]

    # group boundaries
    key = (s_core * TILES + s_tile) * 2 + s_half
    nkeys = CORES * TILES * 2
    counts = np.bincount(key, minlength=nkeys)
    starts = np.concatenate([[0], np.cumsum(counts)])

    # per-(tile, half) edge-tile counts, maxed over cores (SPMD uniformity)
    cnt = counts.reshape(CORES, TILES, 2)
    ktiles = -(-cnt // 128)  # ceil div
    K = ktiles.max(axis=0)  # [TILES, 2]
    # every tile must emit at least one matmul so PSUM gets initialized
    for t in range(TILES):
        if K[t, 0] + K[t, 1] == 0:
            K[t, 0] = 1
    KLO = K[:, 0].astype(int)
    KHI = K[:, 1].astype(int)
    CUMLO = np.concatenate([[0], np.cumsum(KLO)]).astype(int)
    CUMHI = np.concatenate([[0], np.cumsum(KHI)]).astype(int)
    KLO_TOT = int(CUMLO[-1])
    KHI_TOT = int(CUMHI[-1])

    pad_row = NPC  # first dead slot of core 0 (lo) / core CORES//2 (hi); any
    # valid row works since padded edges use dst_local == -1

    xT = np.ascontiguousarray(np.asarray(x, dtype=np.float32).T)  # [CIN, N]

    in_maps = []
    for c in range(CORES):
        idx_lo = np.full(KLO_TOT * 128, pad_row, dtype=np.int64)
        dl_lo = np.full((KLO_TOT, 128), -1.0, dtype=np.float32)
        idx_hi = np.full(KHI_TOT * 128, pad_row, dtype=np.int64)
        dl_hi = np.full((KHI_TOT, 128), -1.0, dtype=np.float32)
        for t in range(TILES):
            for h, (idx_s, dl_s, cum) in enumerate(
                ((idx_lo, dl_lo, CUMLO), (idx_hi, dl_hi, CUMHI))
            ):
                k = (c * TILES + t) * 2 + h
                a, b_ = starts[k], starts[k + 1]
                n = b_ - a
                off = cum[t] * 128
                idx_s[off : off + n] = s_rel[a:b_]
                dl_s.reshape(-1)[off : off + n] = s_slot[a:b_]

        # xT shard with zero-padded dead columns
        xs = np.zeros((cfg.CIN, SLOTS), dtype=np.float32)
        xs[:, :NPC] = xT[:, c * NPC : (c + 1) * NPC]

        deg_own = np.ones((128, TILES), dtype=np.float32)
        dv = deg[c * NPC : (c + 1) * NPC]
        pad = np.ones(SLOTS - NPC, dtype=np.float32)
        deg_own[:, :] = np.concatenate([dv, pad]).reshape(TILES, 128).T

        in_maps.append(
            {
                "xT": xs,
                "W1": np.asarray(W1, dtype=np.float32),
                "W2": np.asarray(W2, dtype=np.float32),
                "b1b": np.tile(np.asarray(b1, dtype=np.float32), (128, 1)),
                "b2b": np.tile(np.asarray(b2, dtype=np.float32), (128, 1)),
                "deg_own": deg_own,
                "iota": np.tile(
                    np.arange(128, dtype=np.float32), (128, 1)
                ),
                "ident": np.eye(128, dtype=np.float32),
                "idx_lo": _wrap_idxs(idx_lo),
                "idx_hi": _wrap_idxs(idx_hi),
                "dl_lo": np.ascontiguousarray(dl_lo.T),  # [128, KLO_TOT]
                "dl_hi": np.ascontiguousarray(dl_hi.T),
            }
        )

    meta = dict(KLO=KLO, KHI=KHI, CUMLO=CUMLO, CUMHI=CUMHI,
                KLO_TOT=KLO_TOT, KHI_TOT=KHI_TOT)
    return in_maps, meta


# ----------------------------------------------------------------------------
# device program
# ----------------------------------------------------------------------------
def build_program(cfg, meta):
    import concourse.bacc as bacc
    import concourse.bass as bass
    import concourse.mybir as mybir
    import concourse.tile as tile

    f32 = mybir.dt.float32
    i16 = mybir.dt.int16
    Alu = mybir.AluOpType
    Act = mybir.ActivationFunctionType

    N, SLOTS, TILES, HALF, TOTAL = cfg.N, cfg.SLOTS, cfg.TILES, cfg.HALF, cfg.TOTAL
    CIN, CHID, COUT = cfg.CIN, cfg.CHID, cfg.COUT
    KLO, KHI = meta["KLO"], meta["KHI"]
    CUMLO, CUMHI = meta["CUMLO"], meta["CUMHI"]
    KLO_TOT, KHI_TOT = meta["KLO_TOT"], meta["KHI_TOT"]

    nc = bacc.Bacc(
        "TRN2",
        target_bir_lowering=False,
        debug=False,
        num_devices=cfg.CORES,
    )

    xT_d = nc.dram_tensor("xT", [CIN, SLOTS], f32, kind="ExternalInput")
    W1_d = nc.dram_tensor("W1", [CIN, CHID], f32, kind="ExternalInput")
    W2_d = nc.dram_tensor("W2", [CHID, COUT], f32, kind="ExternalInput")
    b1b_d = nc.dram_tensor("b1b", [128, CHID], f32, kind="ExternalInput")
    b2b_d = nc.dram_tensor("b2b", [128, COUT], f32, kind="ExternalInput")
    deg_d = nc.dram_tensor("deg_own", [128, TILES], f32, kind="ExternalInput")
    iota_d = nc.dram_tensor("iota", [128, 128], f32, kind="ExternalInput")
    ident_d = nc.dram_tensor("ident", [128, 128], f32, kind="ExternalInput")
    idxlo_d = nc.dram_tensor("idx_lo", [128, KLO_TOT * 8], i16, kind="ExternalInput")
    idxhi_d = nc.dram_tensor("idx_hi", [128, KHI_TOT * 8], i16, kind="ExternalInput")
    dllo_d = nc.dram_tensor("dl_lo", [128, KLO_TOT], f32, kind="ExternalInput")
    dlhi_d = nc.dram_tensor("dl_hi", [128, KHI_TOT], f32, kind="ExternalInput")
    z_d = nc.dram_tensor("z", [SLOTS, COUT], f32, kind="ExternalOutput")

    groups = []
    t0 = 0
    while t0 < TILES:
        groups.append((t0, min(t0 + cfg.GROUP, TILES)))
        t0 += cfg.GROUP

    with tile.TileContext(nc) as tc:
        with (
            tc.tile_pool(name="const", bufs=1) as cpool,
            tc.tile_pool(name="tabs", bufs=1, space="DRAM") as dpool,
            tc.tile_pool(name="msg", bufs=2) as mpool,
            tc.tile_pool(name="oh", bufs=4) as ohpool,
            tc.tile_pool(name="work", bufs=3) as wpool,
            tc.tile_pool(name="psMM", bufs=2, space="PSUM") as psMM_pool,
            tc.tile_pool(name="psT", bufs=2, space="PSUM") as psT_pool,
            tc.tile_pool(name="ps3", bufs=2, space="PSUM") as ps3_pool,
        ):
            # ---- load constants / metadata into SBUF ----
            def load(dram, shape, dtype=f32, name=None):
                t_ = cpool.tile(shape, dtype, name=name or dram.name + "_sb")
                nc.sync.dma_start(out=t_[...], in_=dram.ap())
                return t_

            xT_sb = load(xT_d, [CIN, SLOTS])
            W1_sb = load(W1_d, [CIN, CHID])
            W2_sb = load(W2_d, [CHID, COUT])
            b1b_sb = load(b1b_d, [128, CHID])
            b2b_sb = load(b2b_d, [128, COUT])
            deg_sb = load(deg_d, [128, TILES])
            iota_sb = load(iota_d, [128, 128])
            ident_sb = load(ident_d, [128, 128])
            idxlo_sb = load(idxlo_d, [128, KLO_TOT * 8], i16)
            idxhi_sb = load(idxhi_d, [128, KHI_TOT * 8], i16)
            dllo_sb = load(dllo_d, [128, KLO_TOT])
            dlhi_sb = load(dlhi_d, [128, KHI_TOT])

            g1own = cpool.tile([128, TILES, CHID], f32, name="g1own")
            g2own = cpool.tile([128, TILES, COUT], f32, name="g2own")
            zout = cpool.tile([128, TILES, COUT], f32, name="zout")
            dinv = cpool.tile([128, TILES], f32, name="dinv")

            # dinv = 1/sqrt(deg): ACT sqrt then DVE reciprocal
            sq = wpool.tile([128, TILES], f32, name="sqdeg")
            nc.scalar.sqrt(sq[...], deg_sb[...])
            nc.vector.reciprocal(dinv[...], sq[...])

            bounce1 = dpool.tile([SLOTS, CHID], f32, name="bounce1")
            g1_table = dpool.tile([TOTAL, CHID], f32, name="g1_table")
            bounce2 = dpool.tile([SLOTS, COUT], f32, name="bounce2")
            g2_table = dpool.tile([TOTAL, COUT], f32, name="g2_table")

            # ---- phase A: G1 = dinv * (x @ W1) for own nodes ----
            for t in range(TILES):
                psA = psMM_pool.tile([128, CHID], f32, name="psA", tag="ps")
                nc.tensor.matmul(
                    psA[...],
                    xT_sb[:, t * 128 : (t + 1) * 128],
                    W1_sb[...],
                    start=True,
                    stop=True,
                )
                nc.scalar.mul(g1own[:, t, :], psA[...], dinv[:, t : t + 1])
            nc.sync.dma_start(
                out=bounce1[...].rearrange("(t p) f -> p t f", p=128),
                in_=g1own[...],
            )
            nc.gpsimd.collective_compute(
                "AllGather",
                mybir.AluOpType.bypass,
                replica_groups=[list(range(cfg.CORES))],
                ins=[bounce1[...].opt()],
                outs=[g1_table[...].opt()],
            )

            # ---- message-passing layer driver ----
            def layer(table, feat, own, epilogue):
                """gather from `table` ([TOTAL, feat] DRAM), segment-sum per
                dst tile, call epilogue(t, psum)."""
                for (a, b_) in groups:
                    nlo = int(CUMLO[b_] - CUMLO[a])
                    nhi = int(CUMHI[b_] - CUMHI[a])
                    mlo = mpool.tile([128, max(nlo, 1), feat], f32, name="mlo",
                                     tag=f"mlo{feat}")
                    mhi = mpool.tile([128, max(nhi, 1), feat], f32, name="mhi",
                                     tag=f"mhi{feat}")
                    if nlo:
                        nc.gpsimd.dma_gather(
                            mlo[:, :nlo, :],
                            table[0:HALF, :],
                            idxlo_sb[:, CUMLO[a] * 8 : CUMLO[b_] * 8],
                            num_idxs=nlo * 128,
                            num_idxs_reg=nlo * 128,
                            elem_size=feat,
                            single_packet=False,
                        )
                    if nhi:
                        nc.gpsimd.dma_gather(
                            mhi[:, :nhi, :],
                            table[HALF:TOTAL, :],
                            idxhi_sb[:, CUMHI[a] * 8 : CUMHI[b_] * 8],
                            num_idxs=nhi * 128,
                            num_idxs_reg=nhi * 128,
                            elem_size=feat,
                            single_packet=False,
                        )
                    for t in range(a, b_):
                        psum = psMM_pool.tile([128, feat], f32, name="psB", tag="ps")
                        nmm = int(KLO[t] + KHI[t])
                        i = 0
                        for h, (m_, cum, dl_sb) in enumerate(
                            ((mlo, CUMLO, dllo_sb), (mhi, CUMHI, dlhi_sb))
                        ):
                            for k in range(int((KLO, KHI)[h][t])):
                                col = int(cum[t]) + k
                                oh = ohpool.tile([128, 128], f32, name="oh")
                                nc.vector.tensor_scalar(
                                    oh[...],
                                    iota_sb[...],
                                    dl_sb[:, col : col + 1],
                                    None,
                                    Alu.is_equal,
                                )
                                nc.tensor.matmul(
                                    psum[...],
                                    oh[...],
                                    m_[:, col - int(cum[a]), :],
                                    start=(i == 0),
                                    stop=(i == nmm - 1),
                                )
                                i += 1
                        epilogue(t, psum)

            # ---- layer 1 epilogue: z1 = relu(dinv*(acc+g1own)+b1);
            #      g2own = dinv * (z1 @ W2) ----
            def epi1(t, psum):
                t1 = wpool.tile([128, CHID], f32, name="t1")
                nc.vector.tensor_tensor(t1[...], psum[...], g1own[:, t, :], Alu.add)
                z1 = wpool.tile([128, CHID], f32, name="z1")
                nc.vector.scalar_tensor_tensor(
                    z1[...], t1[...], dinv[:, t : t + 1], b1b_sb[...],
                    Alu.mult, Alu.add,
                )
                z1r = wpool.tile([128, CHID], f32, name="z1r")
                nc.scalar.activation(z1r[...], z1[...], Act.Relu)
                psT = psT_pool.tile([128, 128], f32, name="psT")
                nc.tensor.transpose(psT[...], z1r[...], ident_sb[...])
                z1t = wpool.tile([128, CHID], f32, name="z1t")
                nc.vector.tensor_copy(z1t[...], psT[...])
                ps3 = ps3_pool.tile([128, COUT], f32, name="ps3")
                nc.tensor.matmul(ps3[...], z1t[...], W2_sb[...], start=True, stop=True)
                nc.scalar.mul(g2own[:, t, :], ps3[...], dinv[:, t : t + 1])

            layer(g1_table, CHID, g1own, epi1)
            nc.sync.dma_start(
                out=bounce2[...].rearrange("(t p) f -> p t f", p=128),
                in_=g2own[...],
            )
            nc.gpsimd.collective_compute(
                "AllGather",
                mybir.AluOpType.bypass,
                replica_groups=[list(range(cfg.CORES))],
                ins=[bounce2[...].opt()],
                outs=[g2_table[...].opt()],
            )

            # ---- layer 2 epilogue: z = dinv*(acc+g2own)+b2 ----
            def epi2(t, psum):
                t2 = wpool.tile([128, COUT], f32, name="t2")
                nc.vector.tensor_tensor(t2[...], psum[...], g2own[:, t, :], Alu.add)
                nc.vector.scalar_tensor_tensor(
                    zout[:, t, :], t2[...], dinv[:, t : t + 1], b2b_sb[...],
                    Alu.mult, Alu.add,
                )

            layer(g2_table, COUT, g2own, epi2)
            nc.sync.dma_start(
                out=z_d.ap().rearrange("(t p) f -> p t f", p=128),
                in_=zout[...],
            )

    nc.compile()
    return nc


# ----------------------------------------------------------------------------
# entry point
# ----------------------------------------------------------------------------
def run(cfg, x, edge_index, W1, b1, W2, b2, **run_kwargs):
    global LAST_RESULTS
    from concourse.bass_utils import run_bass_kernel_spmd

    in_maps, meta = prep_inputs(cfg, x, edge_index, W1, b1, W2, b2)
    nc = build_program(cfg, meta)
    res = run_bass_kernel_spmd(
        nc, in_maps, core_ids=list(range(cfg.CORES)), **run_kwargs
    )
    LAST_RESULTS = res
    z = np.concatenate(
        [res.results[c]["z"][: cfg.NPC] for c in range(cfg.CORES)], axis=0
    )[: cfg.N]
    return np.ascontiguousarray(z.astype(np.float32))


def kernel(x, edge_index, W1, b1, W2, b2):
    return run(REAL, x, edge_index, W1, b1, W2, b2)
